# revision 2
# baseline (speedup 1.0000x reference)
"""Trainium2 Bass kernel for nn_CharEmbedding (ragged_sequence).

Computation (see reference):
    rep = concat([emb[first], emb[mid].sum(1), emb[last]], -1)   # [U, 3H]
    out = rep @ head_w + head_b                                  # [U, O]
    tok = out[inv_i].reshape(B, L, O); pad time by (1,1)         # [B, L+2, O]

Strategy: fuse everything at token granularity, data-parallel over the
B*L = 32768 output tokens (4096 per core = exactly 2 sequences).  Host
precomputes per-token vocab indices (first/mid/last gathered through
inv_i) as int16 in the SWDGE wrapped layout.  On each core, per
512-token tile:

  1. SWDGE dma_gather x8 (896 rows each, spread round-robin over all 4
     SWDGE queues) from the DRAM emb table, transpose=False: each 512B
     row lands contiguously in one partition.  Row i (stream s, token t;
     i = s*512 + t) -> partition t%128, free block 4s + t//128.
     The gather is DESCRIPTOR-RATE-bound per queue (~6.5ns/descriptor);
     4-queue spread is ~4x faster than one queue.  Gathers from all
     queues write disjoint block slices of one unified SBUF tile.
  2. DVE: pairwise tree-sum of the 12 mid streams in token-major
     layout: 11 tensor_adds of [128, 1024] bf16 (2x perf mode).
  3. PE: 24 transposes ([128,128] identity matmuls) of first/midsum/
     last into PSUM -> feature-major bf16 [128, 2, 512] per stream.
  4. ACT: evacuate transposed PSUM -> SBUF bf16.
  5. PE: per 128-token subtile: bias (K=1 ones x bias matmul) + 6
     K-chunk matmuls vs head W, fp32 PSUM accumulate (512+256 split).
  6. ACT: evacuate out PSUM -> SBUF bf16; DMA store (host upcasts).

Output rows land contiguously; host assembles the [16, 2050, 768]
padded result (pad rows are zeros and never touch the device).
"""

import numpy as np
import ml_dtypes

import concourse.bacc as bacc
import concourse.mybir as mybir
import concourse.tile as tile
from concourse.bass_utils import run_bass_kernel_spmd

BF16 = ml_dtypes.bfloat16

# Problem constants (hardcoded per contract).
VOCAB = 4000
VOCAB_PAD = 4096
U = 30000
M = 12
H = 256
O = 768
B = 16
L = 2048
N_CORES = 8
T_CORE = (B * L) // N_CORES      # 4096 tokens per core
TILE_T = 512                     # tokens per pipeline tile
ROWS_PER_TOK = 2 + M             # 14 gathered rows per token
KCH = (3 * H) // 128             # 6 K-chunks of the 768-dim contraction
NQ = 4                           # SWDGE queues (ucode max)
GSPLIT = 8                       # gathers per tile (2 per queue)

_NC_CACHE = {}


def build_nc(n_tiles=T_CORE // TILE_T, reps=1, variant="full",
             gbufs=2, tbufs=4, obufs=4, nq=NQ, gsplit=GSPLIT):
    """Build (and compile) the per-core Bass module.

    Tokens handled = n_tiles * TILE_T.  All cores run the same program.
    reps > 1 wraps the pipeline in a For_i hardware loop (timing only).
    """
    t_core = n_tiles * TILE_T
    rows_tile = ROWS_PER_TOK * TILE_T          # 7168
    idx_cols = rows_tile // 16                 # 448 idx columns per tile
    rows_part = rows_tile // gsplit            # 896
    part_cols = idx_cols // gsplit             # 56
    blk_part = rows_part // 128                # 7
    TB = TILE_T // 128                         # 4 token blocks per tile
    assert rows_part % 128 == 0

    nc = bacc.Bacc("TRN2", target_bir_lowering=False, debug=False,
                   num_swdge_queues=nq)

    tbl_d = nc.dram_tensor("tbl", [VOCAB_PAD, H], mybir.dt.bfloat16,
                           kind="ExternalInput")
    wts_d = nc.dram_tensor("wts", [128, KCH * O], mybir.dt.bfloat16,
                           kind="ExternalInput")
    bias_d = nc.dram_tensor("bias", [1, O], mybir.dt.bfloat16,
                            kind="ExternalInput")
    ident_d = nc.dram_tensor("ident", [128, 128], mybir.dt.bfloat16,
                             kind="ExternalInput")
    idx_d = nc.dram_tensor("idx", [128, n_tiles * idx_cols], mybir.dt.int16,
                           kind="ExternalInput")
    out_d = nc.dram_tensor("out", [t_core, O], mybir.dt.bfloat16,
                           kind="ExternalOutput")

    with tile.TileContext(nc) as tc:
        with (
            tc.tile_pool(name="const", bufs=1) as cpool,
            tc.tile_pool(name="gath", bufs=gbufs) as gpool,
            tc.tile_pool(name="mids", bufs=2) as mpool,
            tc.tile_pool(name="trs", bufs=tbufs) as tpool,
            tc.tile_pool(name="outs", bufs=obufs) as opool,
            tc.tile_pool(name="tpsum", bufs=1, space="PSUM") as tppool,
            tc.tile_pool(name="papsum", bufs=2, space="PSUM") as papool,
            tc.tile_pool(name="pbpsum", bufs=2, space="PSUM") as pbpool,
        ):
            # ---- resident constants ----
            wts = cpool.tile([128, KCH, O], mybir.dt.bfloat16)
            nc.sync.dma_start(out=wts[:], in_=wts_d[:].rearrange(
                "p (c o) -> p c o", c=KCH))
            bias_t = cpool.tile([1, O], mybir.dt.bfloat16)
            nc.sync.dma_start(out=bias_t[:], in_=bias_d[:])
            ident = cpool.tile([128, 128], mybir.dt.bfloat16)
            nc.sync.dma_start(out=ident[:], in_=ident_d[:])
            idx_t = cpool.tile([128, n_tiles * idx_cols], mybir.dt.int16)
            nc.sync.dma_start(out=idx_t[:], in_=idx_d[:])
            ones_t = cpool.tile([1, 128], mybir.dt.bfloat16)
            nc.vector.memset(ones_t[:], 1.0)

            import contextlib
            rep_ctx = tc.For_i(0, reps, 1) if reps > 1 else contextlib.nullcontext()
            with rep_ctx:
             for t in range(n_tiles):
                # ---- gather: gsplit x 896 rows over nq queues ----
                g = gpool.tile([128, rows_tile // 128, H],
                               mybir.dt.bfloat16, name="g", tag="g")
                for h in range(gsplit):
                    col0 = t * idx_cols + h * part_cols
                    nc.gpsimd.dma_gather(
                        g[:, h * blk_part:(h + 1) * blk_part, :], tbl_d[:],
                        idx_t[:, col0:col0 + part_cols],
                        rows_part, rows_part, H,
                        transpose=False, single_packet=False,
                        queue_num=h % nq)

                if variant == "gather_only":
                    continue

                # stream s occupies blocks TB*s : TB*s+TB of g
                def mid(j):
                    b0 = TB * (1 + j)
                    return g[:, b0:b0 + TB, :]

                # ---- mid-sum tree on DVE (token-major, bf16 2x) ----
                msA = mpool.tile([128, 6, TB, H], mybir.dt.bfloat16)
                for k in range(6):
                    nc.vector.tensor_add(msA[:, k], mid(2 * k), mid(2 * k + 1))
                msB = mpool.tile([128, 3, TB, H], mybir.dt.bfloat16)
                for k in range(3):
                    nc.vector.tensor_add(msB[:, k], msA[:, 2 * k], msA[:, 2 * k + 1])
                msum = mpool.tile([128, TB, H], mybir.dt.bfloat16)
                nc.vector.tensor_add(msum[:], msB[:, 0], msB[:, 1])
                nc.vector.tensor_add(msum[:], msum[:], msB[:, 2])

                streams = (g[:, 0:TB, :], msum[:], g[:, 13 * TB:14 * TB, :])

                # ---- PE transposes -> feature-major bf16 chunks ----
                trs = []
                for s, src in enumerate(streams):
                    ps = tppool.tile([128, 2, TILE_T], mybir.dt.bfloat16,
                                     name=f"tp{s}", tag=f"tp{s}")
                    for c in range(2):
                        for b in range(TB):
                            nc.tensor.transpose(
                                ps[:, c, b * 128:(b + 1) * 128],
                                src[:, b, c * 128:(c + 1) * 128],
                                ident[:])
                    sb = tpool.tile([128, 2, TILE_T], mybir.dt.bfloat16,
                                    name=f"tr{s}", tag=f"tr{s}")
                    nc.scalar.copy(sb[:], ps[:])
                    trs.append(sb)

                # ---- head matmul + evacuate + store per 128-token subtile ----
                for m in range(TB):
                    tok = slice(m * 128, (m + 1) * 128)
                    ps_a = papool.tile([128, 512], mybir.dt.float32,
                                       name="ps_a", tag="ps_a")
                    ps_b = pbpool.tile([128, 256], mybir.dt.float32,
                                       name="ps_b", tag="ps_b")
                    for ps, osl in ((ps_a, slice(0, 512)), (ps_b, slice(512, O))):
                        nc.tensor.matmul(ps[:], ones_t[:], bias_t[:, osl],
                                         start=True, stop=False)
                        for c in range(KCH):
                            nc.tensor.matmul(
                                ps[:], trs[c // 2][:, c % 2, tok],
                                wts[:, c, osl],
                                start=False, stop=(c == KCH - 1))
                    o_sb = opool.tile([128, O], mybir.dt.bfloat16)
                    nc.scalar.copy(o_sb[:, 0:512], ps_a[:])
                    nc.scalar.copy(o_sb[:, 512:O], ps_b[:])
                    row = t * TILE_T + m * 128
                    nc.sync.dma_start(out=out_d[row:row + 128, :], in_=o_sb[:])

    nc.compile()
    return nc


def _get_nc():
    if "full" not in _NC_CACHE:
        _NC_CACHE["full"] = build_nc()
    return _NC_CACHE["full"]


def _wrap_idx(stream):
    """Pack an index stream into the SWDGE gather layout: idx i lives at
    [i % 16, i // 16], replicated across the 8 groups of 16 partitions."""
    n = stream.shape[0]
    arr = stream.reshape(n // 16, 16).T.astype(np.int16)   # [16, n//16]
    return np.tile(arr, (8, 1))                            # [128, n//16]


def prep_inputs(emb_table, head_w, head_b, first, mid, last, inv_i,
                n_tiles=T_CORE // TILE_T):
    """Host-side shard + layout prep.  Returns in_maps for 8 cores."""
    emb = np.asarray(emb_table, dtype=np.float32).copy()
    emb[0] = 0.0  # padding_idx (reference masks id 0; row 0 is zero anyway)
    tbl16 = np.zeros((VOCAB_PAD, H), dtype=BF16)
    tbl16[:VOCAB] = emb.astype(BF16)

    Wb = np.asarray(head_w, dtype=np.float32).astype(BF16)      # [768, 768]
    wts_in = np.ascontiguousarray(
        Wb.reshape(KCH, 128, O).transpose(1, 0, 2)).reshape(128, KCH * O)
    bias_in = np.asarray(head_b, dtype=np.float32).astype(BF16).reshape(1, O)
    ident_in = np.eye(128, dtype=BF16)

    inv_i = np.asarray(inv_i)
    fi = np.asarray(first)[inv_i].astype(np.int16)   # [B*L]
    mi = np.asarray(mid)[inv_i].astype(np.int16)     # [B*L, 12]
    la = np.asarray(last)[inv_i].astype(np.int16)    # [B*L]

    in_maps = []
    for c in range(N_CORES):
        base = c * T_CORE
        cols = []
        for t in range(n_tiles):
            s = slice(base + t * TILE_T, base + (t + 1) * TILE_T)
            stream = np.concatenate(
                [fi[s]] + [mi[s, j] for j in range(M)] + [la[s]])
            cols.append(_wrap_idx(stream))
        idx_in = np.concatenate(cols, axis=1)
        in_maps.append({
            "tbl": tbl16, "wts": wts_in, "bias": bias_in,
            "ident": ident_in, "idx": idx_in,
        })
    return in_maps


def kernel(emb_table, head_w, head_b, first, mid, last, inv_i,
           batch, seq_len, _nc=None, _return_raw=False):
    batch = int(batch)
    seq_len = int(seq_len)
    assert batch == B and seq_len == L, (batch, seq_len)
    nc = _nc if _nc is not None else _get_nc()
    in_maps = prep_inputs(emb_table, head_w, head_b, first, mid, last, inv_i)
    res = run_bass_kernel_spmd(nc, in_maps, core_ids=list(range(N_CORES)))
    per_core = [r["out"] for r in res.results]         # each [4096, 768] bf16
    if _return_raw:
        return per_core
    full = np.zeros((B, L + 2, O), dtype=np.float32)
    seq_per_core = T_CORE // L                         # 2 sequences per core
    for c in range(N_CORES):
        full[c * seq_per_core:(c + 1) * seq_per_core, 1:L + 1, :] = (
            per_core[c].reshape(seq_per_core, L, O).astype(np.float32))
    return full


# revision 3
# speedup vs baseline: 1.8545x; 1.8545x over previous
"""Trainium2 Bass kernel for nn_CharEmbedding (ragged_sequence).

Computation (see reference):
    rep = concat([emb[first], emb[mid].sum(1), emb[last]], -1)   # [U, 3H]
    out = rep @ head_w + head_b                                  # [U, O]
    tok = out[inv_i].reshape(B, L, O); pad time by (1,1)         # [B, L+2, O]

Strategy: fuse everything at token granularity, data-parallel over the
B*L = 32768 output tokens (4096 per core = exactly 2 sequences).  Host
precomputes per-token vocab indices (first/mid/last gathered through
inv_i) as int16 in the SWDGE wrapped layout.  On each core, per
512-token tile:

  1. SWDGE dma_gather x4 (896 rows each, one per SWDGE queue)
     from the DRAM emb table, transpose=False: each 512B
     row lands contiguously in one partition.  Row i (stream s, token t;
     i = s*512 + t) -> partition t%128, free block 4s + t//128.
     The gather is DESCRIPTOR-RATE-bound per queue (~6.5ns/descriptor);
     4-queue spread is ~4x faster than one queue.  Gathers from all
     queues write disjoint block slices of one unified SBUF tile.
  2. DVE: pairwise tree-sum of the 12 mid streams in token-major
     layout: 11 tensor_adds of [128, 1024] bf16 (2x perf mode).
  3. PE: 24 transposes ([128,128] identity matmuls) of first/midsum/
     last into PSUM -> feature-major bf16 [128, 2, 512] per stream.
  4. ACT: evacuate transposed PSUM -> SBUF bf16.
  5. PE: per 128-token subtile: bias (K=1 ones x bias matmul) + 6
     K-chunk matmuls vs head W, fp32 PSUM accumulate (512+256 split).
  6. ACT: evacuate out PSUM -> SBUF bf16; DMA store (host upcasts).

Output rows land contiguously; host assembles the [16, 2050, 768]
padded result (pad rows are zeros and never touch the device).
"""

import numpy as np
import ml_dtypes

import concourse.bacc as bacc
import concourse.mybir as mybir
import concourse.tile as tile
from concourse.bass_utils import run_bass_kernel_spmd

BF16 = ml_dtypes.bfloat16

# Problem constants (hardcoded per contract).
VOCAB = 4000
VOCAB_PAD = 4096
U = 30000
M = 12
H = 256
O = 768
B = 16
L = 2048
N_CORES = 8
T_CORE = (B * L) // N_CORES      # 4096 tokens per core
TILE_T = 256                     # tokens per pipeline tile
ROWS_PER_TOK = 2 + M             # 14 gathered rows per token
KCH = (3 * H) // 128             # 6 K-chunks of the 768-dim contraction
NQ = 4                           # SWDGE queues (ucode max)
GSPLIT = 4                       # gathers per tile (1 per queue)

_NC_CACHE = {}


def build_nc(n_tiles=T_CORE // TILE_T, reps=1, variant="full",
             gbufs=3, tbufs=4, obufs=4, nq=NQ, gsplit=GSPLIT):
    """Build (and compile) the per-core Bass module.

    Tokens handled = n_tiles * TILE_T.  All cores run the same program.
    reps > 1 wraps the pipeline in a For_i hardware loop (timing only).
    """
    t_core = n_tiles * TILE_T
    rows_tile = ROWS_PER_TOK * TILE_T          # 3584
    idx_cols = rows_tile // 16                 # 224 idx columns per tile
    rows_part = rows_tile // gsplit            # 896
    part_cols = idx_cols // gsplit             # 56
    blk_part = rows_part // 128                # 7
    TB = TILE_T // 128                         # 2 token blocks per tile
    assert rows_part % 128 == 0

    nc = bacc.Bacc("TRN2", target_bir_lowering=False, debug=False,
                   num_swdge_queues=nq)

    tbl_d = nc.dram_tensor("tbl", [VOCAB_PAD, H], mybir.dt.bfloat16,
                           kind="ExternalInput")
    wts_d = nc.dram_tensor("wts", [128, KCH * O], mybir.dt.bfloat16,
                           kind="ExternalInput")
    bias_d = nc.dram_tensor("bias", [1, O], mybir.dt.bfloat16,
                            kind="ExternalInput")
    ident_d = nc.dram_tensor("ident", [128, 128], mybir.dt.bfloat16,
                             kind="ExternalInput")
    idx_d = nc.dram_tensor("idx", [128, n_tiles * idx_cols], mybir.dt.int16,
                           kind="ExternalInput")
    out_d = nc.dram_tensor("out", [t_core, O], mybir.dt.bfloat16,
                           kind="ExternalOutput")

    with tile.TileContext(nc) as tc:
        with (
            tc.tile_pool(name="const", bufs=1) as cpool,
            tc.tile_pool(name="gath", bufs=gbufs) as gpool,
            tc.tile_pool(name="mids", bufs=2) as mpool,
            tc.tile_pool(name="trs", bufs=tbufs) as tpool,
            tc.tile_pool(name="outs", bufs=obufs) as opool,
            tc.tile_pool(name="tpsum", bufs=1, space="PSUM") as tppool,
            tc.tile_pool(name="papsum", bufs=2, space="PSUM") as papool,
            tc.tile_pool(name="pbpsum", bufs=2, space="PSUM") as pbpool,
        ):
            # ---- resident constants ----
            wts = cpool.tile([128, KCH, O], mybir.dt.bfloat16)
            nc.sync.dma_start(out=wts[:], in_=wts_d[:].rearrange(
                "p (c o) -> p c o", c=KCH))
            bias_t = cpool.tile([1, O], mybir.dt.bfloat16)
            nc.sync.dma_start(out=bias_t[:], in_=bias_d[:])
            ident = cpool.tile([128, 128], mybir.dt.bfloat16)
            nc.sync.dma_start(out=ident[:], in_=ident_d[:])
            idx_t = cpool.tile([128, n_tiles * idx_cols], mybir.dt.int16)
            nc.sync.dma_start(out=idx_t[:], in_=idx_d[:])
            ones_t = cpool.tile([1, 128], mybir.dt.bfloat16)
            nc.vector.memset(ones_t[:], 1.0)

            import contextlib
            rep_ctx = tc.For_i(0, reps, 1) if reps > 1 else contextlib.nullcontext()
            with rep_ctx:
             for t in range(n_tiles):
                # ---- gather: gsplit x 896 rows over nq queues ----
                g = gpool.tile([128, rows_tile // 128, H],
                               mybir.dt.bfloat16, name="g", tag="g")
                for h in range(gsplit):
                    col0 = t * idx_cols + h * part_cols
                    nc.gpsimd.dma_gather(
                        g[:, h * blk_part:(h + 1) * blk_part, :], tbl_d[:],
                        idx_t[:, col0:col0 + part_cols],
                        rows_part, rows_part, H,
                        transpose=False, single_packet=False,
                        queue_num=h % nq)

                if variant == "gather_only":
                    continue

                # stream s occupies blocks TB*s : TB*s+TB of g
                def mid(j):
                    b0 = TB * (1 + j)
                    return g[:, b0:b0 + TB, :]

                # ---- mid-sum tree on DVE (token-major, bf16 2x) ----
                msA = mpool.tile([128, 6, TB, H], mybir.dt.bfloat16)
                for k in range(6):
                    nc.vector.tensor_add(msA[:, k], mid(2 * k), mid(2 * k + 1))
                msB = mpool.tile([128, 3, TB, H], mybir.dt.bfloat16)
                for k in range(3):
                    nc.vector.tensor_add(msB[:, k], msA[:, 2 * k], msA[:, 2 * k + 1])
                msum = mpool.tile([128, TB, H], mybir.dt.bfloat16)
                nc.vector.tensor_add(msum[:], msB[:, 0], msB[:, 1])
                nc.vector.tensor_add(msum[:], msum[:], msB[:, 2])

                streams = (g[:, 0:TB, :], msum[:], g[:, 13 * TB:14 * TB, :])

                # ---- PE transposes -> feature-major bf16 chunks ----
                trs = []
                for s, src in enumerate(streams):
                    ps = tppool.tile([128, 2, TILE_T], mybir.dt.bfloat16,
                                     name=f"tp{s}", tag=f"tp{s}")
                    for c in range(2):
                        for b in range(TB):
                            nc.tensor.transpose(
                                ps[:, c, b * 128:(b + 1) * 128],
                                src[:, b, c * 128:(c + 1) * 128],
                                ident[:])
                    sb = tpool.tile([128, 2, TILE_T], mybir.dt.bfloat16,
                                    name=f"tr{s}", tag=f"tr{s}")
                    nc.scalar.copy(sb[:], ps[:])
                    trs.append(sb)

                # ---- head matmul + evacuate + store per 128-token subtile ----
                for m in range(TB):
                    tok = slice(m * 128, (m + 1) * 128)
                    ps_a = papool.tile([128, 512], mybir.dt.float32,
                                       name="ps_a", tag="ps_a")
                    ps_b = pbpool.tile([128, 256], mybir.dt.float32,
                                       name="ps_b", tag="ps_b")
                    for ps, osl in ((ps_a, slice(0, 512)), (ps_b, slice(512, O))):
                        nc.tensor.matmul(ps[:], ones_t[:], bias_t[:, osl],
                                         start=True, stop=False)
                        for c in range(KCH):
                            nc.tensor.matmul(
                                ps[:], trs[c // 2][:, c % 2, tok],
                                wts[:, c, osl],
                                start=False, stop=(c == KCH - 1))
                    o_sb = opool.tile([128, O], mybir.dt.bfloat16)
                    nc.scalar.copy(o_sb[:, 0:512], ps_a[:])
                    nc.scalar.copy(o_sb[:, 512:O], ps_b[:])
                    row = t * TILE_T + m * 128
                    nc.sync.dma_start(out=out_d[row:row + 128, :], in_=o_sb[:])

    nc.compile()
    return nc


def _get_nc():
    if "full" not in _NC_CACHE:
        _NC_CACHE["full"] = build_nc()
    return _NC_CACHE["full"]


def _wrap_idx(stream):
    """Pack an index stream into the SWDGE gather layout: idx i lives at
    [i % 16, i // 16], replicated across the 8 groups of 16 partitions."""
    n = stream.shape[0]
    arr = stream.reshape(n // 16, 16).T.astype(np.int16)   # [16, n//16]
    return np.tile(arr, (8, 1))                            # [128, n//16]


def prep_inputs(emb_table, head_w, head_b, first, mid, last, inv_i,
                n_tiles=T_CORE // TILE_T):
    """Host-side shard + layout prep.  Returns in_maps for 8 cores."""
    emb = np.asarray(emb_table, dtype=np.float32).copy()
    emb[0] = 0.0  # padding_idx (reference masks id 0; row 0 is zero anyway)
    tbl16 = np.zeros((VOCAB_PAD, H), dtype=BF16)
    tbl16[:VOCAB] = emb.astype(BF16)

    Wb = np.asarray(head_w, dtype=np.float32).astype(BF16)      # [768, 768]
    wts_in = np.ascontiguousarray(
        Wb.reshape(KCH, 128, O).transpose(1, 0, 2)).reshape(128, KCH * O)
    bias_in = np.asarray(head_b, dtype=np.float32).astype(BF16).reshape(1, O)
    ident_in = np.eye(128, dtype=BF16)

    inv_i = np.asarray(inv_i)
    fi = np.asarray(first)[inv_i].astype(np.int16)   # [B*L]
    mi = np.asarray(mid)[inv_i].astype(np.int16)     # [B*L, 12]
    la = np.asarray(last)[inv_i].astype(np.int16)    # [B*L]

    in_maps = []
    for c in range(N_CORES):
        base = c * T_CORE
        cols = []
        for t in range(n_tiles):
            s = slice(base + t * TILE_T, base + (t + 1) * TILE_T)
            stream = np.concatenate(
                [fi[s]] + [mi[s, j] for j in range(M)] + [la[s]])
            cols.append(_wrap_idx(stream))
        idx_in = np.concatenate(cols, axis=1)
        in_maps.append({
            "tbl": tbl16, "wts": wts_in, "bias": bias_in,
            "ident": ident_in, "idx": idx_in,
        })
    return in_maps


def kernel(emb_table, head_w, head_b, first, mid, last, inv_i,
           batch, seq_len, _nc=None, _return_raw=False):
    batch = int(batch)
    seq_len = int(seq_len)
    assert batch == B and seq_len == L, (batch, seq_len)
    nc = _nc if _nc is not None else _get_nc()
    in_maps = prep_inputs(emb_table, head_w, head_b, first, mid, last, inv_i)
    res = run_bass_kernel_spmd(nc, in_maps, core_ids=list(range(N_CORES)))
    per_core = [r["out"] for r in res.results]         # each [4096, 768] bf16
    if _return_raw:
        return per_core
    full = np.zeros((B, L + 2, O), dtype=np.float32)
    seq_per_core = T_CORE // L                         # 2 sequences per core
    for c in range(N_CORES):
        full[c * seq_per_core:(c + 1) * seq_per_core, 1:L + 1, :] = (
            per_core[c].reshape(seq_per_core, L, O).astype(np.float32))
    return full
